# revision 3
# baseline (speedup 1.0000x reference)
"""Multi-head causal attention (B=8, S=2048, E=512, H=8, D=64) on 8 trn2 cores.

v3: head-PAIR processing with row-tiled (64x128-mode) scores matmuls: the two
heads of a group run concurrently on PE row tiles T0/T8 (measured 2x), exp'd
by one wide ACTIVATE over a [128, 2, 768] PSUM pair tile (pa, 3 banks).
AV stays K=128 (full rate; LDWEIGHTS hidden). x enters via SWDGE cast DMA to
a bf16 DRAM scratch then HW xbar transpose DMAs straight into xT (no PE
transposes, no DVE staging copies). All matmul operands bf16. Softmax
normalization: batched reciprocal via DRAM partition-spread (as before), then
the 1/l rows are partition-broadcast by DMA re-reads of rscr into [128, SW]
rbc tiles so one DVE multiply normalizes a whole head pair. PE warmed with
dummy matmuls from t=0 so the HAM clock gate reaches 8/8 before real work.
PSUM: pa(3) + po(2x2) + fil(1) = 8 banks; all qkv/proj fillers are 1-bank
[128,512] units interleaved into the pair pipeline between groups.
"""

import os
import numpy as np
from contextlib import ExitStack

K2STOP = int(os.environ.get("K2STOP", "9"))

import concourse.bass as bass
import concourse.mybir as mybir
from concourse.tile import TileContext
from concourse import bass_utils

F32 = mybir.dt.float32
BF16 = mybir.dt.bfloat16
B, S, E, H, D = 8, 2048, 512, 8, 64
P = 128
G = H // 2      # head pairs
EB = E // P     # e blocks
SB = S // P     # s blocks
SW = 1024       # stripe width
NS = S // SW    # stripes
GW = 512        # scores group width; [P,2,GW] = 2 banks, one per head
                # (heads must NOT share a PSUM bank: concurrent row-tile
                # writes to one bank are a HW fatal)
EXP = mybir.ActivationFunctionType.Exp


def build_attention_nc():
    nc = bass.Bass(trn_type="TRN2")

    x_d = nc.dram_tensor("x", [S, E], F32, kind="ExternalInput").ap()
    wq_d = nc.dram_tensor("Wq", [H, E, D], F32, kind="ExternalInput").ap()
    wk_d = nc.dram_tensor("Wk", [H, E, D], F32, kind="ExternalInput").ap()
    wv_d = nc.dram_tensor("Wv", [H, E, D], F32, kind="ExternalInput").ap()
    wp_d = nc.dram_tensor("Wp", [E, E], F32, kind="ExternalInput").ap()
    bp_d = nc.dram_tensor("bp", [E], F32, kind="ExternalInput").ap()
    out_d = nc.dram_tensor("out", [S, E], F32, kind="ExternalOutput").ap()
    xscr = nc.dram_tensor("xscr", [S, E], BF16, kind="Internal").ap()
    lscr = nc.dram_tensor("lscr", [NS * H * SW], BF16, kind="Internal").ap()
    rscr = nc.dram_tensor("rscr", [NS * H * SW], BF16, kind="Internal").ap()

    with TileContext(nc) as tc, ExitStack() as top:
        const = top.enter_context(tc.tile_pool(name="const", bufs=1))
        warm = const.tile([P, 512], BF16, tag="warm")
        wsc = const.tile([1, 2], F32, tag="wsc")
        bpb = const.tile([P, E], F32, tag="bpb")
        wp_sb = const.tile([P, G, E], BF16, tag="wp")

        persist = top.enter_context(tc.tile_pool(name="persist", bufs=1))
        xT = persist.tile([P, EB, S], BF16, tag="xT")
        wq_sb = persist.tile([P, EB, H, D], BF16, tag="wq")
        wk_sb = persist.tile([P, EB, H, D], BF16, tag="wk")
        wv_sb = persist.tile([P, EB, H, D], BF16, tag="wv")

        wstage = top.enter_context(tc.tile_pool(name="wstage", bufs=2))
        qkv = top.enter_context(tc.tile_pool(name="qkv", bufs=1))
        qT = [qkv.tile([P, S], BF16, tag=f"qT{g}", name=f"qT{g}") for g in range(G)]
        kT = [qkv.tile([P, S], BF16, tag=f"kT{g}", name=f"kT{g}") for g in range(G)]
        vt = qkv.tile([P, H, SB, 65], BF16, tag="vt")
        # col 64 = 1.0: the AV matmul's stationary [*, 65] also produces the
        # softmax denominator in output row 64
        nc.vector.memset(vt[:, :, :, 64:65], 1.0)

        attp = top.enter_context(tc.tile_pool(name="attp", bufs=8))
        attout = top.enter_context(tc.tile_pool(name="attout", bufs=1))
        attoutT = [attout.tile([P, G, SW], BF16, tag=f"attoutT{t}",
                               name=f"attoutT{t}") for t in range(NS)]
        lrow = [attout.tile([1, H * SW], BF16, tag=f"lrow{t}", name=f"lrow{t}")
                for t in range(NS)]
        rbcp = top.enter_context(tc.tile_pool(name="rbcp", bufs=2))
        rpool = top.enter_context(tc.tile_pool(name="rp", bufs=2))
        outp = top.enter_context(tc.tile_pool(name="outp", bufs=2))

        # PSUM: pa [P,2,GW] = 2 banks x1; po [65,SW] = 2 banks x2;
        # fil [P,512] = 1 bank x2  -> 8 banks
        ppool = top.enter_context(tc.tile_pool(name="pp", bufs=1, space="PSUM"))

        # ---------------- warmup: dummy matmuls so HAM reaches 8/8 early;
        # exp table prewarm off the critical path
        nc.vector.memset(warm, 0.0)
        nc.vector.memset(wsc, 0.0)
        nc.scalar.activation(out=wsc[0:1, 1:2], in_=wsc[0:1, 0:1], func=EXP)
        pw = ppool.tile([P, 512], F32, tag="pa", name="pwarm", bufs=2)
        for i in range(140):
            nc.tensor.matmul(pw, lhsT=warm[:, 0:P], rhs=warm[:, 0:512],
                             start=(i == 0), stop=(i == 139))

        # ---------------- input DMAs
        # gpsimd (SWDGE, casts): x f32 -> xscr bf16 in 4 chunks; then wp, bp
        for c in range(4):
            nc.gpsimd.dma_start(out=xscr[c * 512:(c + 1) * 512, :],
                                in_=x_d[c * 512:(c + 1) * 512, :])
        for g in range(G):
            nc.gpsimd.dma_start(out=wp_sb[:, g, :],
                                in_=wp_d[g * P:(g + 1) * P, :])
        nc.gpsimd.dma_start(
            out=bpb,
            in_=bass.AP(tensor=bp_d.tensor, offset=bp_d.offset,
                        ap=[[0, P]] + list(bp_d.ap)))

        # weights: raw fp32 strided loads (HWDGE) + DVE cast to bf16
        def load_w(eng, wd, wsb, ej):
            ws = wstage.tile([P, H, D], F32, tag="ws", name="ws")
            eng.dma_start(out=ws,
                          in_=wd.rearrange("h e d -> e h d")[ej * P:(ej + 1) * P])
            nc.vector.tensor_copy(out=wsb[:, ej], in_=ws)

        # sync queue: wk+wq first (keeps the ACT queue free for the exp
        # stream); scalar queue: wv. x transposes split across BOTH HWDGE
        # queues in [1024,128] units (issue cost dominates small transposes)
        for ej in range(EB):
            load_w(nc.sync, wk_d, wk_sb, ej)
            load_w(nc.sync, wq_d, wq_sb, ej)
        for ej in range(EB):
            load_w(nc.scalar, wv_d, wv_sb, ej)
        for hc in range(2):
            for eb in range(EB):
                nc.sync.dma_start_transpose(
                    out=xT[:, eb, hc * 1024:(hc + 1) * 1024],
                    in_=xscr[hc * 1024:(hc + 1) * 1024,
                             eb * P:(eb + 1) * P])

        # ---------------- filler units (each ~1 PSUM bank, ~0.9us PE)
        def emit_v(si, tag="pa"):
            pv = ppool.tile([P, 512], F32, tag=tag, name="pv",
                            bufs=2 if tag == "pa" else 1)
            for ej in range(EB):
                nc.tensor.matmul(pv, lhsT=xT[:, ej, si * P:(si + 1) * P],
                                 rhs=wv_sb[:, ej], start=(ej == 0),
                                 stop=(ej == EB - 1))
            nc.vector.tensor_copy(out=vt[:, :, si, 0:64],
                                  in_=pv.rearrange("p (h d) -> p h d", h=H))

        def emit_qkt(g, c0, which, tag="pa"):
            w_sb, dst = (wk_sb, kT[g]) if which == "k" else (wq_sb, qT[g])
            pq = ppool.tile([P, 512], F32, tag=tag, name="pq",
                            bufs=2 if tag == "pa" else 1)
            for ej in range(EB):
                nc.tensor.matmul(pq, lhsT=w_sb[:, ej, 2 * g:2 * g + 2, :],
                                 rhs=xT[:, ej, c0:c0 + 512],
                                 start=(ej == 0), stop=(ej == EB - 1))
            nc.vector.tensor_copy(out=dst[:, c0:c0 + 512], in_=pq)

        proj_pp = {}

        def emit_proj(si, glo=0, ghi=G, tag="pa"):
            tt, col = si * P // SW, (si * P) % SW
            if glo == 0:
                proj_pp[si] = ppool.tile([P, E], F32, tag=tag, name="ppj",
                                         bufs=2 if tag == "pa" else 1)
            pp = proj_pp[si]
            for g in range(glo, ghi):
                nc.tensor.matmul(pp, lhsT=attoutT[tt][:, g, col:col + P],
                                 rhs=wp_sb[:, g, :], start=(g == 0),
                                 stop=(g == G - 1))
            if ghi == G:
                del proj_pp[si]
                ob = outp.tile([P, E], F32, tag="ob", name="ob")
                nc.vector.tensor_add(out=ob, in0=pp, in1=bpb)
                nc.scalar.dma_start(out=out_d[si * P:(si + 1) * P, :], in_=ob)

        # ---------------- softmax denominator reciprocal (batched via DRAM
        # partition-spread), and the 1/l partition-broadcast + normalize
        def emit_stripe_recip(t, h0, h1):
            n = (h1 - h0) * SW
            off = t * H * SW + h0 * SW
            nc.sync.dma_start(
                out=bass.AP(tensor=lscr.tensor, offset=lscr.offset + off,
                            ap=[[0, 1], [1, n]]),
                in_=lrow[t][0:1, h0 * SW:h1 * SW])
            lsp = rpool.tile([P, n // P], BF16, tag="lsp", name="lsp")
            nc.sync.dma_start(
                out=lsp,
                in_=bass.AP(tensor=lscr.tensor, offset=lscr.offset + off,
                            ap=[[n // P, P], [1, n // P]]))
            rsp = rpool.tile([P, n // P], BF16, tag="rsp", name="rsp")
            with nc.allow_low_precision("softmax denom reciprocal; rel-err "
                                        "budget 2e-2 >> bf16 eps"):
                nc.vector.reciprocal(out=rsp, in_=lsp)
            nc.sync.dma_start(
                out=bass.AP(tensor=rscr.tensor, offset=rscr.offset + off,
                            ap=[[n // P, P], [1, n // P]]),
                in_=rsp)

        def emit_norm(t, g):
            # rbc[0:64] = bcast 1/l(head 2g), rbc[64:128] = bcast 1/l(2g+1)
            rbc = rbcp.tile([P, SW], BF16, tag="rbc", name="rbc")
            for par in range(2):
                off = t * H * SW + (2 * g + par) * SW
                nc.sync.dma_start(
                    out=rbc[par * D:(par + 1) * D, :],
                    in_=bass.AP(tensor=rscr.tensor, offset=rscr.offset + off,
                                ap=[[0, D], [1, SW]]))
            sl = attoutT[t][:, g, :]
            nc.vector.tensor_mul(out=sl, in0=sl, in1=rbc)

        # ---------------- attention pair (heads 2g, 2g+1 on row tiles T0/T8)
        def emit_att_pair(t, g, fillers=()):
            lo, hi = t * SW, (t + 1) * SW
            jmax = hi // P
            fillers = list(fillers)
            # walk the concatenated (j, col) space and cut a group every GW
            # cols: groups are mostly exactly GW wide, pieces are the
            # j-extent intersections with each group window
            groups, cur, w = [], [], 0
            for j in range(jmax):
                cs = max(lo, j * P)
                while cs < hi:
                    ce = min(hi, cs + (GW - w))
                    cur.append((j, cs, ce))
                    w += ce - cs
                    cs = ce
                    if w == GW:
                        groups.append(cur)
                        cur, w = [], 0
            if cur:
                groups.append(cur)

            poA = ppool.tile([65, SW], F32, tag="po", name="poA")
            poB = ppool.tile([65, SW], F32, tag="poB", name="poB")
            po = (poA, poB)
            pending = []
            started_banks = set()

            def consume(pend):
                sbt, par, off, j, cs, ce = pend
                h = 2 * g + par
                # AV chunks split at po bank boundary (col 512 of stripe);
                # start exactly on the first touch of each po bank
                c = cs
                while c < ce:
                    cb = min(ce, lo + ((c - lo) // 512 + 1) * 512)
                    jlast = (cb - 1) // P
                    bank = (par, (c - lo) // 512)
                    nc.tensor.matmul(
                        po[par][:, c - lo:cb - lo],
                        lhsT=vt[:, h, j, :],
                        rhs=sbt[:, par, off + c - cs:off + cb - cs],
                        start=(bank not in started_banks), stop=(j == jlast))
                    started_banks.add(bank)
                    c = cb

            for gi, grp in enumerate(groups):
                if fillers and gi >= 1:
                    fillers.pop(0)()
                pa = ppool.tile([P, 2, GW], F32, tag="pa", name="pa", bufs=2)
                gw = sum(ce - cs for (_, cs, ce) in grp)
                for par in range(2):
                    off = 0
                    for (j, cs, ce) in grp:
                        # scores MM chunks split at pa bank boundaries
                        c = cs
                        while c < ce:
                            toff = par * GW + off + (c - cs)
                            cb = min(ce, c + 512 - (toff % 512))
                            nc.tensor.matmul(
                                pa[:, par, off + c - cs:off + cb - cs],
                                lhsT=kT[g][par * D:(par + 1) * D,
                                           j * P:(j + 1) * P],
                                rhs=qT[g][par * D:(par + 1) * D, c:cb],
                                start=True, stop=True)
                            c = cb
                        off += ce - cs
                while len(pending) >= 5:
                    consume(pending.pop(0))
                sbt = attp.tile([P, 2, GW], BF16, tag="attsb", name="sbt")
                if gw == GW:
                    # full group: [P, 2, GW] is contiguous -> one ACTIVATE
                    # covers both heads
                    nc.scalar.activation(
                        out=sbt, in_=pa,
                        func=EXP, scale=float(1.0 / np.sqrt(D)))
                else:
                    for par in range(2):
                        nc.scalar.activation(
                            out=sbt[:, par, 0:gw], in_=pa[:, par, 0:gw],
                            func=EXP, scale=float(1.0 / np.sqrt(D)))
                off = 0
                for (j, cs, ce) in grp:
                    # causal mask for the diagonal block's upper triangle
                    ds_, de = max(cs, j * P), min(ce, j * P + P)
                    if ds_ < de and j * P >= lo:
                        for par in range(2):
                            for dc in range(ds_, de, P):
                                dce = min(de, dc + P)
                                nc.gpsimd.affine_select(
                                    out=sbt[:, par, off + dc - cs:off + dce - cs],
                                    in_=sbt[:, par, off + dc - cs:off + dce - cs],
                                    compare_op=mybir.AluOpType.is_ge, fill=0.0,
                                    base=dc - j * P,
                                    pattern=[[1, dce - dc]],
                                    channel_multiplier=-1)
                    for par in range(2):
                        pending.append((sbt, par, off, j, cs, ce))
                    off += ce - cs
            # leftover fillers BEFORE the drain: a vf() leftover writes vt
            # data the drained AVs depend on — after the drain it would
            # deadlock the PE queue (AV ahead of its producer on the same
            # strict-FIFO engine)
            for f in fillers:
                f()
            while pending:
                consume(pending.pop(0))
            # stage outputs: attoutT rows 0:64 (even head, plus l_even in row
            # 64 temporarily via the 65-row copy), l rows, odd head
            hh = 0
            nc.vector.tensor_copy(out=attoutT[t][0:65, g, :], in_=poA[0:65, :])
            nc.vector.tensor_copy(out=lrow[t][0:1, (2 * g) * SW:(2 * g + 1) * SW],
                                  in_=attoutT[t][64:65, g, :])
            nc.vector.tensor_copy(out=attoutT[t][64:128, g, :], in_=poB[0:64, :])
            nc.vector.tensor_copy(
                out=lrow[t][0:1, (2 * g + 1) * SW:(2 * g + 2) * SW],
                in_=poB[64:65, :])

        # filler closures
        def qk(g, c0, w):
            return lambda: emit_qkt(g, c0, w)

        def vf(si):
            return lambda: emit_v(si)

        def pf(si):
            return lambda: emit_proj(si)

        def nf(t, g):
            return lambda: emit_norm(t, g)

        def rf(t, h0, h1):
            return lambda: emit_stripe_recip(t, h0, h1)

        # bisect aid: always write the full output first so partial builds
        # still produce fetchable results
        if K2STOP < 9:
            zb = outp.tile([P, E], F32, tag="ob", name="zb")
            nc.vector.memset(zb, 0.0)
            for si in range(SB):
                nc.scalar.dma_start(out=out_d[si * P:(si + 1) * P, :], in_=zb)

        # ---------------- schedule. Prologue rotates PSUM tags so the three
        # 1-bank slots pipeline (po/poB free until the first AV).
        emit_qkt(0, 0, "k", tag="po")
        emit_qkt(0, 0, "q", tag="poB")
        emit_qkt(0, 512, "k", tag="pa")
        emit_qkt(0, 512, "q", tag="po")
        emit_v(0, tag="poB")
        emit_v(1, tag="pa")

        if K2STOP >= 2:
            emit_att_pair(0, 0, [vf(2), vf(3), vf(4), vf(5), vf(6), vf(7),
                                 qk(1, 0, "k"), qk(1, 0, "q"),
                                 qk(1, 512, "k"), qk(1, 512, "q")])
            emit_att_pair(0, 1, [qk(2, 0, "k"), qk(2, 0, "q"),
                                 qk(2, 512, "k"), qk(2, 512, "q")])
            emit_att_pair(0, 2, [qk(3, 0, "k"), qk(3, 0, "q"),
                                 qk(3, 512, "k"), qk(3, 512, "q")])
            emit_att_pair(0, 3, [qk(0, 1024, "k"), qk(0, 1024, "q"),
                                 qk(0, 1536, "k"), qk(0, 1536, "q"),
                                 vf(8), vf(9)])
        if K2STOP >= 3:
            full = K2STOP >= 9
            emit_att_pair(1, 0, [vf(10), vf(11), vf(12), vf(13), vf(14),
                                 vf(15),
                                 qk(1, 1024, "k"), qk(1, 1024, "q"),
                                 qk(1, 1536, "k"), qk(1, 1536, "q")]
                          + ([rf(0, 0, 4)] if full else []))
            emit_att_pair(1, 1, [qk(2, 1024, "k"), qk(2, 1024, "q"),
                                 qk(2, 1536, "k"), qk(2, 1536, "q")]
                          + ([rf(0, 4, 8), nf(0, 0), nf(0, 1)] if full else [])
                          + [qk(3, 1024, "k"), qk(3, 1024, "q")])
            emit_att_pair(1, 2, [qk(3, 1536, "k"), qk(3, 1536, "q")]
                          + ([nf(0, 2), nf(0, 3), pf(0), pf(1), pf(2), pf(3),
                              rf(1, 0, 2)] if full else []))
            emit_att_pair(1, 3, [pf(4), pf(5), pf(6), pf(7),
                                 rf(1, 2, 6), nf(1, 0), nf(1, 1), nf(1, 2)]
                          if full else [])
        if K2STOP >= 9:
            # tail: overlap heads-6,7 reciprocal with partial projections of
            # stripe-1 rows (heads 0-5 already normalized)
            emit_stripe_recip(1, 6, 8)
            emit_proj(8, 0, 3, tag="pa")
            emit_proj(9, 0, 3, tag="pa")
            emit_proj(10, 0, 3, tag="po")
            emit_proj(11, 0, 3, tag="poB")
            emit_norm(1, 3)
            for si in (8, 9, 10, 11):
                emit_proj(si, 3, G)
            for si in range(12, SB):
                emit_proj(si)

    _hoist_matmul_waits(nc)
    return nc


def _hoist_matmul_waits(nc):
    """Several TRN2 ISA structs accept only one sync-wait slot; hoist every
    wait of a multi-wait instruction onto same-engine NoOps inserted right
    before it (same engine queue => identical ordering semantics)."""
    nid = [0]
    for fn in nc.m.functions:
        for blk in fn.blocks:
            insts = blk.instructions
            out = []
            for inst in insts:
                si = inst.sync_info
                if (inst.engine != mybir.EngineType.Unassigned and si is not None
                        and len(si.on_wait) >= 2 and inst.opcode != "NoOp"):
                    for w in si.on_wait:
                        nid[0] += 1
                        nop = mybir.InstNoOp(name=f"I-mmwait-{nid[0]}",
                                             ins=[], outs=[])
                        nop.engine = inst.engine
                        nop.sync_info = mybir.SyncInfo(on_wait=[w], on_update=[])
                        nc.inst_map[nop.name] = nop
                        out.append(nop)
                    inst.sync_info = mybir.SyncInfo(on_wait=[],
                                                    on_update=list(si.on_update))
                out.append(inst)
            if len(out) != len(insts):
                insts[:] = out


_nc_cache = {}


def _get_nc():
    if "nc" not in _nc_cache:
        _nc_cache["nc"] = build_attention_nc()
    return _nc_cache["nc"]


def kernel(x, Wq, Wk, Wv, Wp, bp, _trace=False):
    nc = _get_nc()
    n = x.shape[0]
    wq = np.ascontiguousarray(Wq, np.float32)
    wk = np.ascontiguousarray(Wk, np.float32)
    wv = np.ascontiguousarray(Wv, np.float32)
    wp = np.ascontiguousarray(Wp, np.float32)
    bpc = np.ascontiguousarray(bp, np.float32)
    in_maps = [
        {"x": np.ascontiguousarray(x[b], np.float32),
         "Wq": wq, "Wk": wk, "Wv": wv, "Wp": wp, "bp": bpc}
        for b in range(n)
    ]
    res = bass_utils.run_bass_kernel_spmd(
        nc, in_maps, core_ids=list(range(n)), trace=_trace)
    out = np.stack([r["out"] for r in res.results], axis=0)
    if _trace:
        return out, res
    return out
